# revision 11
# baseline (speedup 1.0000x reference)
"""Trainium2 Bass kernel for nn_CensoredLoss_Sub.

reference:
    out = outputs.reshape(B, T, D)                     # D = 2
    loss1 = targets[:, :, 0:1] * log((1 - out) + eps)
    loss2 = targets[:, :, 1:2] * log(out + eps)
    loss  = sum((loss1 + loss2) * weights[:, :, None], axis=(0, 1))  # (D,)
    return -loss / (B * T)

Strategy: pure data-parallel over B across 8 cores; per-core partial sums
are gathered and reduced on host (the (D,)=2-float all-reduce is trivial).

The kernel was HBM-bound at f32, so inputs are stored compactly in DRAM:
  - t, w as bf16. The compute path always ran t/w products in bf16;
    storing bf16 is numerically identical and halves those bytes.
  - o as fp16 pre-scaled by C = 1-2^-11. fp16 keeps the 10-bit mantissa
    that log(1-o) needs near o->1 (bf16 rounds ~0.2% of o to exactly 1.0
    -> log(0)); the pre-scale keeps fp16(o*C) strictly below 1.0 so the
    descale inside ACT's affine (scale=S~=1/C) never reproduces exactly
    1.0 and 1-o' stays positive. Measured end-to-end rel err ~7e-4.
With no DMA-time dtype casts left, all loads go through HWDGE
(nc.sync.dma_start): ~420 GB/s measured, no Q7 involvement.

At 10.5 MB/core the stream fits under the ACT (ScalarE) floor — 2 Ln
evaluations per o-element at 1 elem/cycle/lane = ~31 us — so the kernel
is ACT-paced: everything is arranged so ACT starts as early as possible
and never stalls:
  - DMA order keeps o one tile ahead of tw (ACT only reads o; DVE's
    tw-dependent work can lag).
  - Tile sizes make the ACT cadence per tile (2*(224+F)/1.2 ns) exceed
    the DMA cadence (2.5*F bytes / ~420 GB/s), so after the first tile
    ACT is the pipeline pacer.
  - First/last tiles are small: ACT starts ~0.6 us after first byte, and
    the post-ACT drain (last products + matmuls + psum copies) is short.

Key identity: for both d=0,1 the coefficient of log(1-o_d) is w*t0 and
the coefficient of log(o_d+eps) is w*t1:
    loss_d = sum_pairs  (w*t0)*log(1-o_d) + (w*t1)*log(o_d+eps)

Host-side layout (pure permutation + dtype cast, no arithmetic on
values): per tile, o is deinterleaved into [o0|o1] and t/w are packed
into one [t0|t1|w] block, so every on-chip access is contiguous (strided
APs break DVE 2x packing).

Per tile (SF o-elems, SP=SF/2 pairs per partition):
  ACT:  l1 = Ln(1 - S*o'), l2 = Ln(S*o' + eps)      (bf16 out)
  DVE:  xy = [t0|t1]*w (bf16 2x); p1 = x*l1, p2 = y*l2 per parity
  PE:   ones[128,1]^T @ product-chunks into 4 psum chains
        (p1/p2 x d0/d1, [1,512] each) - the p1 chains close one DVE op
        before the p2 chains, so their psum->SBUF copies overlap the
        final p2 work.
Final: ACT/DVE copy the 4 psums to SBUF, DMA [1,2048] out; host sums
per-core partials and applies -1/(B*T).
"""

import numpy as np

B, T, D = 16384, 512, 2
N_CORES = 8
EPS = 1e-8
P = 128

FO = (B // N_CORES) * T * D // P  # o columns per partition = 16384

# fp16 pre-scale for o: largest fp16(o*C) must stay < 1.0 after the f32
# descale multiply inside ACT. Computed once, deterministically.
O_SCALE = np.float32(1.0 - 2.0 ** -11)
_s = np.float32(1.0) / O_SCALE
while np.float32(np.float16(O_SCALE)) * _s >= np.float32(1.0):
    _s = np.nextafter(_s, np.float32(0.0))
O_DESCALE = float(_s)

# Per-tile o columns; every SF a multiple of 1024 so matmul chunks are
# exactly MM_N wide (each PSUM chain's start/stop covers identical cols).
TILES = [1024, 2048, 3072, 3072, 3072, 2048, 1024, 1024]
assert sum(TILES) == FO
assert all(F % 1024 == 0 for F in TILES)
FMAX = max(TILES)
MM_N = 512  # matmul moving free dim

_compiled = {}


def _build():
    import concourse.mybir as mybir
    from concourse import bacc
    from concourse.tile import TileContext

    f32 = mybir.dt.float32
    f16 = mybir.dt.float16
    bf16 = mybir.dt.bfloat16
    Ln = mybir.ActivationFunctionType.Ln
    Copy = mybir.ActivationFunctionType.Copy

    nc = bacc.Bacc(
        "TRN2",
        target_bir_lowering=False,
        debug=False,
        num_devices=N_CORES,
    )
    o_d = nc.dram_tensor("o", [P, FO], f16, kind="ExternalInput").ap()
    tw_d = nc.dram_tensor("tw", [P, FO + FO // 2], bf16, kind="ExternalInput").ap()
    acc_d = nc.dram_tensor("acc", [1, 4 * MM_N], f32, kind="ExternalOutput").ap()

    n_tiles = len(TILES)

    with TileContext(nc) as tc:
        with (
            tc.tile_pool(name="io", bufs=4) as io_pool,
            tc.tile_pool(name="mid", bufs=4) as mid_pool,
            tc.tile_pool(name="lp", bufs=3) as l_pool,
            tc.tile_pool(name="one", bufs=1) as one_pool,
            tc.tile_pool(name="ps", bufs=1, space="PSUM") as psum_pool,
        ):
            bias_eps = one_pool.tile([P, 1], f32)
            bias_one = one_pool.tile([P, 1], f32)
            ones = one_pool.tile([P, 1], bf16)
            res = one_pool.tile([1, 4 * MM_N], f32)
            nc.vector.memset(bias_eps[:], EPS)
            nc.vector.memset(bias_one[:], 1.0)
            nc.vector.memset(ones[:], 1.0)
            # 4 independent psum chains: (p1,p2) x (d0,d1)
            psum = [
                psum_pool.tile([1, MM_N], f32, tag=f"ps{k}", name=f"psum{k}")
                for k in range(4)
            ]
            dummy = one_pool.tile([P, 1], bf16)
            # warm the Ln table set while the first DMA is in flight
            nc.scalar.activation(dummy[:], bias_eps[:], Ln, bias=bias_one[:], scale=1.0)

            # HWDGE loads, one FIFO ring; o runs one tile ahead of tw so
            # ACT (the pacer) never waits for tw bytes.
            o_offs = [0]
            for F in TILES:
                o_offs.append(o_offs[-1] + F)
            tw_offs = [(v * 3) // 2 for v in o_offs]
            ots = [
                io_pool.tile([P, FMAX], f16, tag="ot", name=f"ot{g}")
                for g in range(n_tiles)
            ]
            twts = [
                io_pool.tile([P, 3 * FMAX // 2], bf16, tag="twt", name=f"twt{g}")
                for g in range(n_tiles)
            ]

            def dma_o(g):
                F = TILES[g]
                nc.sync.dma_start(
                    out=ots[g][:, :F], in_=o_d[:, o_offs[g] : o_offs[g] + F]
                )

            def dma_tw(g):
                Fb = 3 * TILES[g] // 2
                nc.sync.dma_start(
                    out=twts[g][:, :Fb],
                    in_=tw_d[:, tw_offs[g] : tw_offs[g] + Fb],
                )

            dma_o(0)
            dma_o(1)
            dma_tw(0)
            for g in range(2, n_tiles):
                dma_o(g)
                dma_tw(g - 1)
            dma_tw(n_tiles - 1)

            for g in range(n_tiles):
                SF = TILES[g]
                SP = SF // 2
                ot = ots[g]
                tw = twts[g][:, : 3 * SP].rearrange("p (c f) -> p c f", c=3)

                l1 = l_pool.tile([P, FMAX], bf16, tag="l1")
                l2 = l_pool.tile([P, FMAX], bf16, tag="l2")
                # l1 = Ln(1 - S*o'), l2 = Ln(S*o' + eps); descale fused in
                nc.scalar.activation(
                    l1[:, :SF], ot[:, :SF], Ln, bias=bias_one[:], scale=-O_DESCALE
                )
                nc.scalar.activation(
                    l2[:, :SF], ot[:, :SF], Ln, bias=bias_eps[:], scale=O_DESCALE
                )

                xy = mid_pool.tile([P, 2, FMAX // 2], bf16, tag="xy")
                p1 = mid_pool.tile([P, FMAX], bf16, tag="p1")
                p2 = mid_pool.tile([P, FMAX], bf16, tag="p2")
                # one TT for [x|y] = [t0|t1] * w_bcast; step-0 middle dim
                # keeps the 2x mode (innermost stays step-1)
                wb = tw[:, 2, :].unsqueeze(1).broadcast_to([P, 2, SP])
                nc.vector.tensor_mul(xy[:, :, :SP], tw[:, 0:2, :], wb)
                # products: x (resp. y) broadcast over both parity halves
                l1v = l1[:, :SF].rearrange("p (d f) -> p d f", d=2)
                l2v = l2[:, :SF].rearrange("p (d f) -> p d f", d=2)
                p1v = p1[:, :SF].rearrange("p (d f) -> p d f", d=2)
                p2v = p2[:, :SF].rearrange("p (d f) -> p d f", d=2)
                xb = xy[:, 0, :SP].unsqueeze(1).broadcast_to([P, 2, SP])
                yb = xy[:, 1, :SP].unsqueeze(1).broadcast_to([P, 2, SP])
                nc.vector.tensor_mul(p1v, xb, l1v)
                nc.vector.tensor_mul(p2v, yb, l2v)
                for pi, prod in enumerate((p1, p2)):
                    for dd in range(2):
                        ps = psum[2 * pi + dd]
                        for c in range(SP // MM_N):
                            nc.tensor.matmul(
                                ps[:],
                                ones[:],
                                prod[:, dd * SP + c * MM_N : dd * SP + (c + 1) * MM_N],
                                start=(g == 0 and c == 0),
                                stop=(g == n_tiles - 1 and c == SP // MM_N - 1),
                            )

            # p1 chains close one DVE op before p2's: ACT (free after its
            # last Ln) copies them while the final p2 matmuls run, and the
            # first half of the output ships early.
            nc.scalar.activation(res[:, 0:MM_N], psum[0][:], Copy, bias=0.0, scale=1.0)
            nc.scalar.activation(
                res[:, MM_N : 2 * MM_N], psum[1][:], Copy, bias=0.0, scale=1.0
            )
            nc.sync.dma_start(out=acc_d[:, : 2 * MM_N], in_=res[:, : 2 * MM_N])
            nc.scalar.activation(
                res[:, 2 * MM_N : 3 * MM_N], psum[2][:], Copy, bias=0.0, scale=1.0
            )
            nc.vector.tensor_copy(res[:, 3 * MM_N : 4 * MM_N], psum[3][:])
            nc.sync.dma_start(
                out=acc_d[:, 2 * MM_N :], in_=res[:, 2 * MM_N : 4 * MM_N]
            )
    nc.compile()
    return nc


def _get_nc():
    if "nc" not in _compiled:
        _compiled["nc"] = _build()
    return _compiled["nc"]


def _deint(x2d):
    """[P, FO] interleaved -> per-tile [d0-block | d1-block] layout."""
    out = np.empty_like(x2d)
    off = 0
    for F in TILES:
        v = x2d[:, off : off + F].reshape(P, F // 2, 2).transpose(0, 2, 1)
        out[:, off : off + F] = v.reshape(P, F)
        off += F
    return out


def _to_bf16(x):
    """f32 -> bf16 (round-to-nearest-even) stored as ml_dtypes.bfloat16."""
    import ml_dtypes

    u = x.view(np.uint32)
    rounded = (u + 0x7FFF + ((u >> 16) & 1)) >> 16
    return rounded.astype(np.uint16).view(ml_dtypes.bfloat16)


def _pack_tw(t2d, w2d):
    """Pack [P,FO] t (interleaved) + [P,FO/2] w into per-tile [t0|t1|w]
    blocks -> [P, FO + FO//2] bf16. Permutation + dtype cast only."""
    import ml_dtypes

    out = np.empty((P, FO + FO // 2), dtype=ml_dtypes.bfloat16)
    t_off = w_off = b_off = 0
    tb = _to_bf16(t2d)
    wb = _to_bf16(w2d)
    for F in TILES:
        FP = F // 2
        tv = tb[:, t_off : t_off + F].reshape(P, FP, 2).transpose(0, 2, 1)
        out[:, b_off : b_off + F] = tv.reshape(P, F)
        out[:, b_off + F : b_off + F + FP] = wb[:, w_off : w_off + FP]
        t_off += F
        w_off += FP
        b_off += F + FP
    return out


def make_in_maps(outputs, targets, weights):
    rows = B // N_CORES
    in_maps = []
    for c in range(N_CORES):
        sh = slice(c * rows, (c + 1) * rows)
        o_scaled = (
            np.ascontiguousarray(outputs[sh]).reshape(P, FO) * O_SCALE
        ).astype(np.float16)
        in_maps.append(
            {
                "o": _deint(o_scaled),
                "tw": _pack_tw(
                    np.ascontiguousarray(targets[sh]).reshape(P, FO),
                    np.ascontiguousarray(weights[sh]).reshape(P, FO // 2),
                ),
            }
        )
    return in_maps


def run_raw(in_maps, **kw):
    from concourse import bass_utils

    nc = _get_nc()
    return bass_utils.run_bass_kernel_spmd(
        nc, in_maps, core_ids=list(range(N_CORES)), **kw
    )


def finish(results) -> np.ndarray:
    total = np.zeros(2, dtype=np.float64)
    for r in results:
        a = r["acc"].astype(np.float64).reshape(4, MM_N)
        total[0] += a[0].sum() + a[2].sum()
        total[1] += a[1].sum() + a[3].sum()
    return (-total / (B * T)).astype(np.float32)


def kernel(outputs: np.ndarray, targets: np.ndarray, weights: np.ndarray) -> np.ndarray:
    outputs = np.asarray(outputs, dtype=np.float32)
    targets = np.asarray(targets, dtype=np.float32)
    weights = np.asarray(weights, dtype=np.float32)
    res = run_raw(make_in_maps(outputs, targets, weights))
    return finish(res.results)


# revision 14
# speedup vs baseline: 1.0856x; 1.0856x over previous
"""Trainium2 Bass kernel for nn_CensoredLoss_Sub.

reference:
    out = outputs.reshape(B, T, D)                     # D = 2
    loss1 = targets[:, :, 0:1] * log((1 - out) + eps)
    loss2 = targets[:, :, 1:2] * log(out + eps)
    loss  = sum((loss1 + loss2) * weights[:, :, None], axis=(0, 1))  # (D,)
    return -loss / (B * T)

Strategy: pure data-parallel over B across 8 cores; per-core partial sums
are gathered and reduced on host (the (D,)=2-float all-reduce is trivial).

The kernel was HBM-bound at f32, so inputs are stored compactly in DRAM:
  - t, w as bf16. The compute path always ran t/w products in bf16;
    storing bf16 is numerically identical and halves those bytes.
  - o as fp16 pre-scaled by C = 1-2^-11. fp16 keeps the 10-bit mantissa
    that log(1-o) needs near o->1 (bf16 rounds ~0.2% of o to exactly 1.0
    -> log(0)); the pre-scale keeps fp16(o*C) strictly below 1.0 so the
    descale inside ACT's affine (scale=S~=1/C) never reproduces exactly
    1.0 and 1-o' stays positive. Measured end-to-end rel err ~7e-4.
With no DMA-time dtype casts left, all loads go through HWDGE
(nc.sync.dma_start): ~420 GB/s measured, no Q7 involvement.

At 10.5 MB/core the stream fits under the ACT (ScalarE) floor — 2 Ln
evaluations per o-element at 1 elem/cycle/lane = ~31 us — so the kernel
is ACT-paced: everything is arranged so ACT starts as early as possible
and never stalls:
  - DMA order keeps o one tile ahead of tw (ACT only reads o; DVE's
    tw-dependent work can lag).
  - Tile sizes make the ACT cadence per tile (2*(224+F)/1.2 ns) exceed
    the DMA cadence (2.5*F bytes / ~420 GB/s), so after the first tile
    ACT is the pipeline pacer.
  - First/last tiles are small: ACT starts ~0.6 us after first byte, and
    the post-ACT drain (last products + matmuls + psum copies) is short.

Key identity: for both d=0,1 the coefficient of log(1-o_d) is w*t0 and
the coefficient of log(o_d+eps) is w*t1:
    loss_d = sum_pairs  (w*t0)*log(1-o_d) + (w*t1)*log(o_d+eps)

Host-side layout (pure permutation + dtype cast, no arithmetic on
values): per tile, o is deinterleaved into [o0|o1] and t/w are packed
into one [t0|t1|w] block, so every on-chip access is contiguous (strided
APs break DVE 2x packing).

Per tile (SF o-elems, SP=SF/2 pairs per partition):
  ACT:  l1 = Ln(1 - S*o'), l2 = Ln(S*o' + eps)      (bf16 out)
  DVE:  xy = [t0|t1]*w (bf16 2x); p1 = x*l1, p2 = y*l2 per parity
  PE:   ones[128,1]^T @ product-chunks into 4 psum chains
        (p1/p2 x d0/d1, [1,512] each) - the p1 chains close one DVE op
        before the p2 chains, so their psum->SBUF copies overlap the
        final p2 work.
Final: ACT/DVE copy the 4 psums to SBUF, DMA [1,2048] out; host sums
per-core partials and applies -1/(B*T).
"""

import numpy as np

B, T, D = 16384, 512, 2
N_CORES = 8
EPS = 1e-8
P = 128

FO = (B // N_CORES) * T * D // P  # o columns per partition = 16384

# fp16 pre-scale for o: largest fp16(o*C) must stay < 1.0 after the f32
# descale multiply inside ACT. Computed once, deterministically.
O_SCALE = np.float32(1.0 - 2.0 ** -11)
_s = np.float32(1.0) / O_SCALE
while np.float32(np.float16(O_SCALE)) * _s >= np.float32(1.0):
    _s = np.nextafter(_s, np.float32(0.0))
O_DESCALE = float(_s)

# Per-tile o columns; every SF a multiple of 1024 so matmul chunks are
# exactly MM_N wide (each PSUM chain's start/stop covers identical cols).
TILES = [1024, 3072, 3072, 3072, 3072, 1024, 1024, 1024]
assert sum(TILES) == FO
assert all(F % 1024 == 0 for F in TILES)
FMAX = max(TILES)
MM_N = 512  # matmul moving free dim

_compiled = {}


def _build():
    import concourse.mybir as mybir
    from concourse import bacc
    from concourse.tile import TileContext

    f32 = mybir.dt.float32
    f16 = mybir.dt.float16
    bf16 = mybir.dt.bfloat16
    Ln = mybir.ActivationFunctionType.Ln
    Copy = mybir.ActivationFunctionType.Copy

    nc = bacc.Bacc(
        "TRN2",
        target_bir_lowering=False,
        debug=False,
        num_devices=N_CORES,
    )
    o_d = nc.dram_tensor("o", [P, FO], f16, kind="ExternalInput").ap()
    tw_d = nc.dram_tensor("tw", [P, FO + FO // 2], bf16, kind="ExternalInput").ap()
    acc_d = nc.dram_tensor("acc", [1, 4 * MM_N], f32, kind="ExternalOutput").ap()

    n_tiles = len(TILES)

    with TileContext(nc) as tc:
        with (
            tc.tile_pool(name="io", bufs=3) as io_pool,
            tc.tile_pool(name="mid", bufs=6) as mid_pool,
            tc.tile_pool(name="lp", bufs=3) as l_pool,
            tc.tile_pool(name="one", bufs=1) as one_pool,
            tc.tile_pool(name="ps", bufs=1, space="PSUM") as psum_pool,
        ):
            bias_eps = one_pool.tile([P, 1], f32)
            bias_one = one_pool.tile([P, 1], f32)
            ones = one_pool.tile([P, 1], bf16)
            res = one_pool.tile([1, 4 * MM_N], f32)
            nc.vector.memset(bias_eps[:], EPS)
            nc.vector.memset(bias_one[:], 1.0)
            nc.vector.memset(ones[:], 1.0)
            # 4 independent psum chains: (p1,p2) x (d0,d1)
            psum = [
                psum_pool.tile([1, MM_N], f32, tag=f"ps{k}", name=f"psum{k}")
                for k in range(4)
            ]
            dummy = one_pool.tile([P, 1], bf16)
            # warm the Ln table set while the first DMA is in flight
            nc.scalar.activation(dummy[:], bias_eps[:], Ln, bias=bias_one[:], scale=1.0)

            # HWDGE loads, one FIFO ring; o runs one tile ahead of tw so
            # ACT (the pacer) never waits for tw bytes.
            o_offs = [0]
            for F in TILES:
                o_offs.append(o_offs[-1] + F)
            tw_offs = [(v * 3) // 2 for v in o_offs]
            ots = [
                io_pool.tile([P, FMAX], f16, tag="ot", name=f"ot{g}")
                for g in range(n_tiles)
            ]
            twts = [
                io_pool.tile([P, 3 * FMAX // 2], bf16, tag="twt", name=f"twt{g}")
                for g in range(n_tiles)
            ]

            def dma_o(g):
                F = TILES[g]
                nc.sync.dma_start(
                    out=ots[g][:, :F], in_=o_d[:, o_offs[g] : o_offs[g] + F]
                )

            def dma_tw(g):
                Fb = 3 * TILES[g] // 2
                nc.sync.dma_start(
                    out=twts[g][:, :Fb],
                    in_=tw_d[:, tw_offs[g] : tw_offs[g] + Fb],
                )

            dma_o(0)
            dma_o(1)
            dma_tw(0)
            for g in range(2, n_tiles):
                dma_o(g)
                dma_tw(g - 1)
            dma_tw(n_tiles - 1)

            for g in range(n_tiles):
                SF = TILES[g]
                SP = SF // 2
                ot = ots[g]
                tw = twts[g][:, : 3 * SP].rearrange("p (c f) -> p c f", c=3)

                l1 = l_pool.tile([P, FMAX], bf16, tag="l1")
                l2 = l_pool.tile([P, FMAX], bf16, tag="l2")
                # l1 = Ln(1 - S*o'), l2 = Ln(S*o' + eps); descale fused in
                nc.scalar.activation(
                    l1[:, :SF], ot[:, :SF], Ln, bias=bias_one[:], scale=-O_DESCALE
                )
                nc.scalar.activation(
                    l2[:, :SF], ot[:, :SF], Ln, bias=bias_eps[:], scale=O_DESCALE
                )

                xy = mid_pool.tile([P, 2, FMAX // 2], bf16, tag="xy")
                p1 = mid_pool.tile([P, FMAX], bf16, tag="p1")
                p2 = mid_pool.tile([P, FMAX], bf16, tag="p2")
                # one TT for [x|y] = [t0|t1] * w_bcast; step-0 middle dim
                # keeps the 2x mode (innermost stays step-1)
                wb = tw[:, 2, :].unsqueeze(1).broadcast_to([P, 2, SP])
                nc.vector.tensor_mul(xy[:, :, :SP], tw[:, 0:2, :], wb)
                # products: x (resp. y) broadcast over both parity halves
                l1v = l1[:, :SF].rearrange("p (d f) -> p d f", d=2)
                l2v = l2[:, :SF].rearrange("p (d f) -> p d f", d=2)
                p1v = p1[:, :SF].rearrange("p (d f) -> p d f", d=2)
                p2v = p2[:, :SF].rearrange("p (d f) -> p d f", d=2)
                xb = xy[:, 0, :SP].unsqueeze(1).broadcast_to([P, 2, SP])
                yb = xy[:, 1, :SP].unsqueeze(1).broadcast_to([P, 2, SP])
                nc.vector.tensor_mul(p1v, xb, l1v)
                nc.vector.tensor_mul(p2v, yb, l2v)
                for pi, prod in enumerate((p1, p2)):
                    for dd in range(2):
                        ps = psum[2 * pi + dd]
                        for c in range(SP // MM_N):
                            nc.tensor.matmul(
                                ps[:],
                                ones[:],
                                prod[:, dd * SP + c * MM_N : dd * SP + (c + 1) * MM_N],
                                start=(g == 0 and c == 0),
                                stop=(g == n_tiles - 1 and c == SP // MM_N - 1),
                            )

            # p1 chains close one DVE op before p2's: ACT (free after its
            # last Ln) copies them while the final p2 matmuls run, and the
            # first half of the output ships early.
            nc.scalar.activation(res[:, 0:MM_N], psum[0][:], Copy, bias=0.0, scale=1.0)
            nc.scalar.activation(
                res[:, MM_N : 2 * MM_N], psum[1][:], Copy, bias=0.0, scale=1.0
            )
            nc.sync.dma_start(out=acc_d[:, : 2 * MM_N], in_=res[:, : 2 * MM_N])
            nc.scalar.activation(
                res[:, 2 * MM_N : 3 * MM_N], psum[2][:], Copy, bias=0.0, scale=1.0
            )
            nc.vector.tensor_copy(res[:, 3 * MM_N : 4 * MM_N], psum[3][:])
            nc.sync.dma_start(
                out=acc_d[:, 2 * MM_N :], in_=res[:, 2 * MM_N : 4 * MM_N]
            )
    nc.compile()
    return nc


def _get_nc():
    if "nc" not in _compiled:
        _compiled["nc"] = _build()
    return _compiled["nc"]


def _deint(x2d):
    """[P, FO] interleaved -> per-tile [d0-block | d1-block] layout."""
    out = np.empty_like(x2d)
    off = 0
    for F in TILES:
        v = x2d[:, off : off + F].reshape(P, F // 2, 2).transpose(0, 2, 1)
        out[:, off : off + F] = v.reshape(P, F)
        off += F
    return out


def _to_bf16(x):
    """f32 -> bf16 (round-to-nearest-even) stored as ml_dtypes.bfloat16."""
    import ml_dtypes

    u = x.view(np.uint32)
    rounded = (u + 0x7FFF + ((u >> 16) & 1)) >> 16
    return rounded.astype(np.uint16).view(ml_dtypes.bfloat16)


def _pack_tw(t2d, w2d):
    """Pack [P,FO] t (interleaved) + [P,FO/2] w into per-tile [t0|t1|w]
    blocks -> [P, FO + FO//2] bf16. Permutation + dtype cast only."""
    import ml_dtypes

    out = np.empty((P, FO + FO // 2), dtype=ml_dtypes.bfloat16)
    t_off = w_off = b_off = 0
    tb = _to_bf16(t2d)
    wb = _to_bf16(w2d)
    for F in TILES:
        FP = F // 2
        tv = tb[:, t_off : t_off + F].reshape(P, FP, 2).transpose(0, 2, 1)
        out[:, b_off : b_off + F] = tv.reshape(P, F)
        out[:, b_off + F : b_off + F + FP] = wb[:, w_off : w_off + FP]
        t_off += F
        w_off += FP
        b_off += F + FP
    return out


def make_in_maps(outputs, targets, weights):
    rows = B // N_CORES
    in_maps = []
    for c in range(N_CORES):
        sh = slice(c * rows, (c + 1) * rows)
        o_scaled = (
            np.ascontiguousarray(outputs[sh]).reshape(P, FO) * O_SCALE
        ).astype(np.float16)
        in_maps.append(
            {
                "o": _deint(o_scaled),
                "tw": _pack_tw(
                    np.ascontiguousarray(targets[sh]).reshape(P, FO),
                    np.ascontiguousarray(weights[sh]).reshape(P, FO // 2),
                ),
            }
        )
    return in_maps


def run_raw(in_maps, **kw):
    from concourse import bass_utils

    nc = _get_nc()
    return bass_utils.run_bass_kernel_spmd(
        nc, in_maps, core_ids=list(range(N_CORES)), **kw
    )


def finish(results) -> np.ndarray:
    total = np.zeros(2, dtype=np.float64)
    for r in results:
        a = r["acc"].astype(np.float64).reshape(4, MM_N)
        total[0] += a[0].sum() + a[2].sum()
        total[1] += a[1].sum() + a[3].sum()
    return (-total / (B * T)).astype(np.float32)


def kernel(outputs: np.ndarray, targets: np.ndarray, weights: np.ndarray) -> np.ndarray:
    outputs = np.asarray(outputs, dtype=np.float32)
    targets = np.asarray(targets, dtype=np.float32)
    weights = np.asarray(weights, dtype=np.float32)
    res = run_raw(make_in_maps(outputs, targets, weights))
    return finish(res.results)
